# Initial kernel scaffold
#
"""BinaryNormalizedConv2d on 8 Trainium2 NeuronCores.

Reference computation (per full input):
  Wq = (w > mean(w)), bq = (b > mean(b))          # {0,1} f32
  z  = conv2d(x, Wq, stride 1, pad 1) + bq
  z  = (z - mean_b(z)) / (sqrt(var_b(z, ddof=1)) + 1e-5)   # per-sample over (C,H,W)
  out = relu(z)

Sharding: data-parallel over batch (32 -> 4 per core), weights replicated.

Device kernel (per core, B=4, Cin=128, Cout=256, H=W=56):
  - x stored padded per-sample [Cin=128 partitions, b, 58*58+4] bf16 so each
    conv tap (kh,kw) is a pure flat offset kh*58+kw (weights {0,1} are exact
    in bf16; only x quantization contributes error, ~2e-3 absmax-relative).
  - 10 dummy matmuls on scratch data warm the PE HAM clock gate to 2.4GHz
    while the input DMAs land (w via ACT HWDGE queue, x via SP queue).
  - conv: per (b, cout-half, y-block of 8 rows): 9 accumulating bf16 matmuls
    into one PSUM bank; rhs is a strided 8x58->8x56 AP so N = 448 with no
    garbage columns. Back-to-back issue measured at ~191ns (PE bf16
    roofline, 1 elem/cycle/lane).
  - PSUM evac via VectorE tensor_scalar copy with accum_out producing
    per-channel row sums (keeps ScalarE free so its big ops never gate
    PSUM-bank reuse); sum of squares via ScalarE Square + accum_out.
  - Per-sample stats: partition-reduce + broadcast via ones-matmul; bias is
    folded into the normalize constants (scale = 1/(sqrt(var)+eps),
    bias2[c] = (bq[c] - mean) * scale), so the conv output never needs the
    bias added explicitly.
  - normalize+relu: ScalarE Relu(z*scale + bias2) in 4 chunks per
    (half, sample), each followed by its output DMA on the SP queue, so the
    tail drains while the next sample's conv still runs.
"""

import numpy as np
import ml_dtypes
from contextlib import ExitStack

# ---- problem constants (hardcoded per contract) ----
B_FULL, CIN, H, W = 32, 128, 56, 56
COUT, KK = 256, 3
N_CORES = 8
B = B_FULL // N_CORES          # 4 samples per core
HP = H + 2                     # 58 padded rows/cols
SB_B = HP * HP + 4             # per-sample stride in padded x (3368), slack for tap overrun
YB = 7                         # y-blocks
RPB = H // YB                  # 8 rows per block
NFREE = RPB * HP               # 464 matmul free size
NINT = RPB * W                 # 448 interior elements per block
HW = H * W                     # 3136
NELEM = COUT * HW              # 802816 elements per sample for stats
EPS = 1e-5

_CACHE = {}
TRACE = False                  # set by test.py to collect an NTFF profile
TRACE_DIR = None
LAST_RESULTS = None


def _emit(nc, tc, x_d, w_d, bq_d, y_d):
    import concourse.mybir as mybir

    f32 = mybir.dt.float32
    bf16 = mybir.dt.bfloat16
    AF = mybir.ActivationFunctionType
    OP = mybir.AluOpType
    AX = mybir.AxisListType

    with ExitStack() as ctx:
        const_pool = ctx.enter_context(tc.tile_pool(name="const", bufs=1))
        xpool = ctx.enter_context(tc.tile_pool(name="x", bufs=1))
        zpool = ctx.enter_context(tc.tile_pool(name="z", bufs=5))
        sqpool = ctx.enter_context(tc.tile_pool(name="sq", bufs=3))
        stpool = ctx.enter_context(tc.tile_pool(name="st", bufs=2))
        outpool = ctx.enter_context(tc.tile_pool(name="out", bufs=6))
        cpsum = ctx.enter_context(tc.tile_pool(name="cps", bufs=7, space="PSUM"))
        spsum = ctx.enter_context(tc.tile_pool(name="sps", bufs=1, space="PSUM"))

        w_sb = const_pool.tile([CIN, 2 * 9 * 128], bf16)
        nc.sync.dma_start(w_sb[:, 0:9 * 128], w_d[:, 0:9 * 128])
        nc.sync.dma_start(w_sb[:, 9 * 128:], w_d[:, 9 * 128:])
        bq_sb = const_pool.tile([128, 3], f32)
        nc.sync.dma_start(bq_sb[:], bq_d[:])
        ones = const_pool.tile([128, 128], f32)
        nc.vector.memset(ones[:], 1.0)

        # PE warm-up: dummy matmuls on scratch data while input DMAs land.
        # They ramp the HAM clock gate to 8/8 so real conv starts at 2.4GHz.
        scr = const_pool.tile([128, 576], bf16)
        nc.vector.memset(scr[:], 0.0)
        for _ in range(10):
            dzt = cpsum.tile([128, NINT], f32, tag="zt")
            nc.tensor.matmul(dzt[:], scr[:, 0:128], scr[:, 0:448],
                             start=True, stop=True)

        x_sb = xpool.tile([CIN, B * SB_B], bf16)
        xcut0 = 2 * NFREE + 3 * HP      # rows for yb=0..1 plus halo
        xcut = 4 * NFREE + 2 * HP
        nc.scalar.dma_start(x_sb[:, 0:xcut0], x_d[:, 0:xcut0])
        nc.scalar.dma_start(x_sb[:, xcut0:xcut], x_d[:, xcut0:xcut])
        nc.scalar.dma_start(x_sb[:, xcut:SB_B], x_d[:, xcut:SB_B])
        for b in range(1, B):
            nc.scalar.dma_start(x_sb[:, b * SB_B:(b + 1) * SB_B],
                                x_d[:, b * SB_B:(b + 1) * SB_B])

        taps = [kh * HP + kw for kh in range(3) for kw in range(3)]

        for b in range(B):
            z_hb = []
            rsums = stpool.tile([128, 16], f32, tag="rsums")
            qsums = stpool.tile([128, 16], f32, tag="qsums")
            for h in range(2):
                z_sb = zpool.tile([128, HW], f32, tag="z")
                z_hb.append(z_sb)
                for yb in range(YB):
                    zt = cpsum.tile([128, NINT], f32, tag="zt")
                    zt3 = zt[:].rearrange("p (r c) -> p r c", c=W)
                    o0 = b * SB_B + yb * NFREE
                    for t in range(9):
                        rhs = x_sb[:, o0 + taps[t]: o0 + taps[t] + NFREE].rearrange(
                            "p (r c) -> p r c", c=HP)[:, :, 0:W]
                        nc.tensor.matmul(
                            zt3,
                            w_sb[:, (h * 9 + t) * 128:(h * 9 + t + 1) * 128],
                            rhs,
                            start=(t == 0), stop=(t == 8),
                        )
                    # evac (contiguous now) + per-channel row sums
                    si = h * 7 + yb
                    nc.vector.tensor_scalar(
                        out=z_sb[:, yb * NINT:(yb + 1) * NINT], in0=zt[:],
                        scalar1=1.0, scalar2=0.0,
                        op0=OP.mult, op1=OP.add,
                        accum_out=rsums[:, si:si + 1])
                    sq = sqpool.tile([128, NINT], f32, tag="sq")
                    zslice = z_sb[:, yb * NINT:(yb + 1) * NINT]
                    if b == B - 1:
                        # last sample: sumsq on VectorE so the stats matmul
                        # isn't gated by ACT's FIFO (prev sample's Relus)
                        nc.vector.scalar_tensor_tensor(
                            out=sq[:], in0=zslice, scalar=1.0, in1=zslice,
                            op0=OP.mult, op1=OP.mult,
                            accum_out=qsums[:, si:si + 1])
                    else:
                        nc.scalar.activation(
                            sq[:], zslice, AF.Square,
                            accum_out=qsums[:, si:si + 1])

            # ---- per-sample stats ----
            stats = stpool.tile([128, 6], f32, tag="stats")
            nc.vector.tensor_reduce(
                stats[:, 0:2],
                rsums[:, 0:14].rearrange("p (h y) -> p h y", y=7),
                axis=AX.X, op=OP.add)
            nc.vector.tensor_reduce(
                stats[:, 2:4],
                qsums[:, 0:14].rearrange("p (h y) -> p h y", y=7),
                axis=AX.X, op=OP.add)
            nc.vector.tensor_tensor(stats[:, 4:6], stats[:, 0:2], bq_sb[:, 0:2],
                                    op=OP.mult)
            st_ps = spsum.tile([128, 6], f32, tag="stps")
            nc.tensor.matmul(st_ps[:], ones[:], stats[:], start=True, stop=True)
            sb_st = stpool.tile([128, 6], f32, tag="sbst")
            nc.vector.tensor_copy(sb_st[:], st_ps[:])

            # scal cols: 0 S, 1 SStot accum, 2 BR*2+C1, 3 Stot, 4 mean, 5 var, 6 inv, 7 tmp
            scal = stpool.tile([128, 8], f32, tag="scal")
            nc.vector.tensor_tensor(scal[:, 0:1], sb_st[:, 0:1], sb_st[:, 1:2], op=OP.add)
            nc.vector.tensor_tensor(scal[:, 1:2], sb_st[:, 2:3], sb_st[:, 3:4], op=OP.add)
            nc.vector.tensor_tensor(scal[:, 2:3], sb_st[:, 4:5], sb_st[:, 5:6], op=OP.add)
            # Stot = S + C1   (C1 = HW * sum(bq), supplied as bq_sb[:, 2])
            nc.vector.tensor_tensor(scal[:, 3:4], scal[:, 0:1], bq_sb[:, 2:3], op=OP.add)
            nc.vector.tensor_scalar_mul(scal[:, 4:5], scal[:, 3:4], 1.0 / NELEM)
            nc.vector.tensor_tensor(scal[:, 7:8], scal[:, 3:4], scal[:, 4:5], op=OP.mult)
            # SStot = Q + 2*BR + C1
            nc.vector.tensor_scalar_mul(scal[:, 2:3], scal[:, 2:3], 2.0)
            nc.vector.tensor_tensor(scal[:, 1:2], scal[:, 1:2], scal[:, 2:3], op=OP.add)
            nc.vector.tensor_tensor(scal[:, 1:2], scal[:, 1:2], bq_sb[:, 2:3], op=OP.add)
            # var = (SStot - Stot*mean) / (n-1)
            nc.vector.tensor_tensor(scal[:, 5:6], scal[:, 1:2], scal[:, 7:8], op=OP.subtract)
            nc.vector.tensor_scalar_mul(scal[:, 5:6], scal[:, 5:6], 1.0 / (NELEM - 1))
            # inv = 1 / (sqrt(var) + eps)
            nc.scalar.sqrt(scal[:, 6:7], scal[:, 5:6])
            nc.vector.tensor_scalar_add(scal[:, 6:7], scal[:, 6:7], EPS)
            nc.vector.reciprocal(scal[:, 6:7], scal[:, 6:7])
            # bias2[:, h] = (bq[:, h] - mean) * inv
            b2 = stpool.tile([128, 2], f32, tag="b2")
            for h in range(2):
                nc.vector.tensor_tensor(b2[:, h:h + 1], bq_sb[:, h:h + 1],
                                        scal[:, 4:5], op=OP.subtract)
                nc.vector.tensor_tensor(b2[:, h:h + 1], b2[:, h:h + 1],
                                        scal[:, 6:7], op=OP.mult)

            # ---- normalize + relu + store (chunked for tail overlap) ----
            HChunk = HW // 4
            for h in range(2):
                for ck in range(4):
                    zn = outpool.tile([128, HChunk], f32, tag="zn")
                    zsrc = z_hb[h][:, ck * HChunk:(ck + 1) * HChunk]
                    nc.scalar.activation(
                        zn[:], zsrc, AF.Relu,
                        bias=b2[:, h:h + 1], scale=scal[:, 6:7])
                    nc.sync.dma_start(
                        y_d[b, h * 128:(h + 1) * 128, ck * HChunk:(ck + 1) * HChunk],
                        zn[:])

def _build_program():
    import concourse.bacc as bacc
    import concourse.tile as tile
    import concourse.mybir as mybir

    f32 = mybir.dt.float32
    bf16 = mybir.dt.bfloat16

    nc = bacc.Bacc("TRN2", target_bir_lowering=False, debug=False, num_devices=1)

    x_d = nc.dram_tensor("x", [CIN, B * SB_B], bf16, kind="ExternalInput").ap()
    w_d = nc.dram_tensor("w", [CIN, 2 * 9 * 128], bf16, kind="ExternalInput").ap()
    bq_d = nc.dram_tensor("bq", [128, 3], f32, kind="ExternalInput").ap()
    y_d = nc.dram_tensor("y", [B, COUT, HW], f32, kind="ExternalOutput").ap()

    with tile.TileContext(nc) as tc:
        _emit(nc, tc, x_d, w_d, bq_d, y_d)

    nc.compile()
    return nc


def _get_program():
    if "nc" not in _CACHE:
        _CACHE["nc"] = _build_program()
    return _CACHE["nc"]


def _binarize(t_np):
    """(t > t.mean()) as f32, matching the reference's jnp computation."""
    try:
        import jax.numpy as jnp
        tj = jnp.asarray(t_np)
        return np.asarray((tj > tj.mean()).astype(jnp.float32))
    except Exception:
        return (t_np > np.float32(t_np.astype(np.float64).mean())).astype(np.float32)


def kernel(x, weight, bias, train_mode=None):
    """Full-input entry point: shards over 8 NeuronCores, returns full output."""
    import time
    last_err = None
    for attempt in range(3):
        try:
            return _kernel_impl(x, weight, bias)
        except Exception as e:  # transient NRT/device errors: back off and retry
            last_err = e
            if attempt < 2:
                time.sleep(20.0 * (attempt + 1))
    raise last_err


def _kernel_impl(x, weight, bias):
    global LAST_RESULTS
    from concourse.bass_utils import run_bass_kernel_spmd

    x = np.asarray(x, dtype=np.float32)
    weight = np.asarray(weight, dtype=np.float32)
    bias = np.asarray(bias, dtype=np.float32)

    wq = _binarize(weight)                       # [256,128,3,3] {0,1}
    bq = _binarize(bias)                         # [256] {0,1}

    # weights -> lhsT layout [ci, (h,t,co_l)]
    wflat = np.ascontiguousarray(
        wq.reshape(2, 128, CIN, 9).transpose(2, 0, 3, 1).reshape(CIN, 2 * 9 * 128)
    ).astype(ml_dtypes.bfloat16)

    bq2 = np.zeros((128, 3), np.float32)
    bq2[:, 0] = bq[0:128]
    bq2[:, 1] = bq[128:256]
    bq2[:, 2] = HW * bq.sum()                    # C1 constant, replicated

    # x -> padded bf16 [b, ci, SB_B]
    xall = np.zeros((B_FULL, CIN, SB_B), dtype=ml_dtypes.bfloat16)
    xv = xall[:, :, :HP * HP].reshape(B_FULL, CIN, HP, HP)
    xv[:, :, 1:H + 1, 1:W + 1] = x.astype(ml_dtypes.bfloat16)

    in_maps = []
    for c in range(N_CORES):
        xc = np.ascontiguousarray(
            xall[c * B:(c + 1) * B].transpose(1, 0, 2).reshape(CIN, B * SB_B))
        in_maps.append({"x": xc, "w": wflat, "bq": bq2})

    nc = _get_program()
    kwargs = {}
    if TRACE:
        kwargs = dict(trace=True, tmpdir=TRACE_DIR)
    res = run_bass_kernel_spmd(nc, in_maps, core_ids=list(range(N_CORES)), **kwargs)
    LAST_RESULTS = res

    out = np.concatenate([res.results[c]["y"] for c in range(N_CORES)], axis=0)
    return out.reshape(B_FULL, COUT, H, W)



# revision 1
# speedup vs baseline: 1.0188x; 1.0188x over previous
"""BinaryNormalizedConv2d on 8 Trainium2 NeuronCores.

Reference computation (per full input):
  Wq = (w > mean(w)), bq = (b > mean(b))          # {0,1} f32
  z  = conv2d(x, Wq, stride 1, pad 1) + bq
  z  = (z - mean_b(z)) / (sqrt(var_b(z, ddof=1)) + 1e-5)   # per-sample over (C,H,W)
  out = relu(z)

Sharding: data-parallel over batch (32 -> 4 per core), weights replicated.

Device kernel (per core, B=4, Cin=128, Cout=256, H=W=56):
  - x stored padded per-sample [Cin=128 partitions, b, 58*58+4] bf16 so each
    conv tap (kh,kw) is a pure flat offset kh*58+kw (weights {0,1} are exact
    in bf16; only x quantization contributes error, ~2e-3 absmax-relative).
  - 10 dummy matmuls on scratch data warm the PE HAM clock gate to 2.4GHz
    while the input DMAs land (w via ACT HWDGE queue, x via SP queue).
  - conv: per (b, cout-half, y-block of 8 rows): 9 accumulating bf16 matmuls
    into one PSUM bank; rhs is a strided 8x58->8x56 AP so N = 448 with no
    garbage columns. Back-to-back issue measured at ~191ns (PE bf16
    roofline, 1 elem/cycle/lane).
  - PSUM evac via VectorE tensor_scalar copy with accum_out producing
    per-channel row sums (keeps ScalarE free so its big ops never gate
    PSUM-bank reuse); sum of squares via ScalarE Square + accum_out.
  - Per-sample stats: partition-reduce + broadcast via ones-matmul; bias is
    folded into the normalize constants (scale = 1/(sqrt(var)+eps),
    bias2[c] = (bq[c] - mean) * scale), so the conv output never needs the
    bias added explicitly.
  - normalize+relu: ScalarE Relu(z*scale + bias2) in 4 chunks per
    (half, sample), each followed by its output DMA on the SP queue, so the
    tail drains while the next sample's conv still runs.
"""

import numpy as np
import ml_dtypes
from contextlib import ExitStack

# ---- problem constants (hardcoded per contract) ----
B_FULL, CIN, H, W = 32, 128, 56, 56
COUT, KK = 256, 3
N_CORES = 8
B = B_FULL // N_CORES          # 4 samples per core
HP = H + 2                     # 58 padded rows/cols
SB_B = HP * HP + 4             # per-sample stride in padded x (3368), slack for tap overrun
YB = 7                         # y-blocks
RPB = H // YB                  # 8 rows per block
NFREE = RPB * HP               # 464 matmul free size
NINT = RPB * W                 # 448 interior elements per block
HW = H * W                     # 3136
NELEM = COUT * HW              # 802816 elements per sample for stats
EPS = 1e-5

_CACHE = {}
TRACE = False                  # set by test.py to collect an NTFF profile
TRACE_DIR = None
LAST_RESULTS = None


def _emit(nc, tc, x_d, w_d, bq_d, y_d):
    import concourse.mybir as mybir

    f32 = mybir.dt.float32
    bf16 = mybir.dt.bfloat16
    AF = mybir.ActivationFunctionType
    OP = mybir.AluOpType
    AX = mybir.AxisListType

    with ExitStack() as ctx:
        const_pool = ctx.enter_context(tc.tile_pool(name="const", bufs=1))
        xpool = ctx.enter_context(tc.tile_pool(name="x", bufs=1))
        zpool = ctx.enter_context(tc.tile_pool(name="z", bufs=5))
        sqpool = ctx.enter_context(tc.tile_pool(name="sq", bufs=3))
        stpool = ctx.enter_context(tc.tile_pool(name="st", bufs=2))
        outpool = ctx.enter_context(tc.tile_pool(name="out", bufs=6))
        cpsum = ctx.enter_context(tc.tile_pool(name="cps", bufs=7, space="PSUM"))
        spsum = ctx.enter_context(tc.tile_pool(name="sps", bufs=1, space="PSUM"))

        w_sb = const_pool.tile([CIN, 2 * 9 * 128], bf16)
        nc.sync.dma_start(w_sb[:, 0:9 * 128], w_d[:, 0:9 * 128])
        nc.sync.dma_start(w_sb[:, 9 * 128:], w_d[:, 9 * 128:])
        bq_sb = const_pool.tile([128, 3], f32)
        nc.sync.dma_start(bq_sb[:], bq_d[:])
        ones = const_pool.tile([128, 128], f32)
        nc.vector.memset(ones[:], 1.0)

        # PE warm-up: dummy matmuls on scratch data while input DMAs land.
        # They ramp the HAM clock gate to 8/8 so real conv starts at 2.4GHz.
        scr = const_pool.tile([128, 576], bf16)
        nc.vector.memset(scr[:], 0.0)
        for _ in range(10):
            dzt = cpsum.tile([128, NINT], f32, tag="zt")
            nc.tensor.matmul(dzt[:], scr[:, 0:128], scr[:, 0:448],
                             start=True, stop=True)

        x_sb = xpool.tile([CIN, B * SB_B], bf16)
        xcut0 = 2 * NFREE + 3 * HP      # rows for yb=0..1 plus halo
        xcut = 4 * NFREE + 2 * HP
        nc.scalar.dma_start(x_sb[:, 0:xcut0], x_d[:, 0:xcut0])
        nc.scalar.dma_start(x_sb[:, xcut0:xcut], x_d[:, xcut0:xcut])
        nc.scalar.dma_start(x_sb[:, xcut:SB_B], x_d[:, xcut:SB_B])
        for b in range(1, B):
            nc.scalar.dma_start(x_sb[:, b * SB_B:(b + 1) * SB_B],
                                x_d[:, b * SB_B:(b + 1) * SB_B])

        taps = [kh * HP + kw for kh in range(3) for kw in range(3)]

        for b in range(B):
            z_hb = []
            rsums = stpool.tile([128, 16], f32, tag="rsums")
            qsums = stpool.tile([128, 16], f32, tag="qsums")
            for h in range(2):
                z_sb = zpool.tile([128, HW], f32, tag="z")
                z_hb.append(z_sb)
                for yb in range(YB):
                    zt = cpsum.tile([128, NINT], f32, tag="zt")
                    zt3 = zt[:].rearrange("p (r c) -> p r c", c=W)
                    o0 = b * SB_B + yb * NFREE
                    for t in range(9):
                        rhs = x_sb[:, o0 + taps[t]: o0 + taps[t] + NFREE].rearrange(
                            "p (r c) -> p r c", c=HP)[:, :, 0:W]
                        nc.tensor.matmul(
                            zt3,
                            w_sb[:, (h * 9 + t) * 128:(h * 9 + t + 1) * 128],
                            rhs,
                            start=(t == 0), stop=(t == 8),
                        )
                    # evac (contiguous now) + per-channel row sums
                    si = h * 7 + yb
                    nc.vector.tensor_scalar(
                        out=z_sb[:, yb * NINT:(yb + 1) * NINT], in0=zt[:],
                        scalar1=1.0, scalar2=0.0,
                        op0=OP.mult, op1=OP.add,
                        accum_out=rsums[:, si:si + 1])
                    sq = sqpool.tile([128, NINT], f32, tag="sq")
                    zslice = z_sb[:, yb * NINT:(yb + 1) * NINT]
                    if b == B - 1:
                        # last sample: sumsq on VectorE so the stats matmul
                        # isn't gated by ACT's FIFO (prev sample's Relus)
                        nc.vector.scalar_tensor_tensor(
                            out=sq[:], in0=zslice, scalar=1.0, in1=zslice,
                            op0=OP.mult, op1=OP.mult,
                            accum_out=qsums[:, si:si + 1])
                    else:
                        nc.scalar.activation(
                            sq[:], zslice, AF.Square,
                            accum_out=qsums[:, si:si + 1])

            # ---- per-sample stats ----
            stats = stpool.tile([128, 6], f32, tag="stats")
            nc.vector.tensor_reduce(
                stats[:, 0:2],
                rsums[:, 0:14].rearrange("p (h y) -> p h y", y=7),
                axis=AX.X, op=OP.add)
            nc.vector.tensor_reduce(
                stats[:, 2:4],
                qsums[:, 0:14].rearrange("p (h y) -> p h y", y=7),
                axis=AX.X, op=OP.add)
            nc.vector.tensor_tensor(stats[:, 4:6], stats[:, 0:2], bq_sb[:, 0:2],
                                    op=OP.mult)
            st_ps = spsum.tile([128, 6], f32, tag="stps")
            nc.tensor.matmul(st_ps[:], ones[:], stats[:], start=True, stop=True)
            sb_st = stpool.tile([128, 6], f32, tag="sbst")
            nc.vector.tensor_copy(sb_st[:], st_ps[:])

            # scal cols: 0 S, 1 SStot accum, 2 BR*2+C1, 3 Stot, 4 mean, 5 var, 6 inv, 7 tmp
            scal = stpool.tile([128, 8], f32, tag="scal")
            nc.vector.tensor_tensor(scal[:, 0:1], sb_st[:, 0:1], sb_st[:, 1:2], op=OP.add)
            nc.vector.tensor_tensor(scal[:, 1:2], sb_st[:, 2:3], sb_st[:, 3:4], op=OP.add)
            nc.vector.tensor_tensor(scal[:, 2:3], sb_st[:, 4:5], sb_st[:, 5:6], op=OP.add)
            # Stot = S + C1   (C1 = HW * sum(bq), supplied as bq_sb[:, 2])
            nc.vector.tensor_tensor(scal[:, 3:4], scal[:, 0:1], bq_sb[:, 2:3], op=OP.add)
            nc.vector.tensor_scalar_mul(scal[:, 4:5], scal[:, 3:4], 1.0 / NELEM)
            nc.vector.tensor_tensor(scal[:, 7:8], scal[:, 3:4], scal[:, 4:5], op=OP.mult)
            # SStot = Q + 2*BR + C1
            nc.vector.tensor_scalar_mul(scal[:, 2:3], scal[:, 2:3], 2.0)
            nc.vector.tensor_tensor(scal[:, 1:2], scal[:, 1:2], scal[:, 2:3], op=OP.add)
            nc.vector.tensor_tensor(scal[:, 1:2], scal[:, 1:2], bq_sb[:, 2:3], op=OP.add)
            # var = (SStot - Stot*mean) / (n-1)
            nc.vector.tensor_tensor(scal[:, 5:6], scal[:, 1:2], scal[:, 7:8], op=OP.subtract)
            nc.vector.tensor_scalar_mul(scal[:, 5:6], scal[:, 5:6], 1.0 / (NELEM - 1))
            # inv = 1 / (sqrt(var) + eps)
            nc.scalar.sqrt(scal[:, 6:7], scal[:, 5:6])
            nc.vector.tensor_scalar_add(scal[:, 6:7], scal[:, 6:7], EPS)
            nc.vector.reciprocal(scal[:, 6:7], scal[:, 6:7])
            # bias2[:, h] = (bq[:, h] - mean) * inv
            b2 = stpool.tile([128, 2], f32, tag="b2")
            for h in range(2):
                nc.vector.tensor_tensor(b2[:, h:h + 1], bq_sb[:, h:h + 1],
                                        scal[:, 4:5], op=OP.subtract)
                nc.vector.tensor_tensor(b2[:, h:h + 1], b2[:, h:h + 1],
                                        scal[:, 6:7], op=OP.mult)

            # ---- normalize + relu + store (chunked for tail overlap) ----
            HChunk = HW // 4
            for h in range(2):
                for ck in range(4):
                    zn = outpool.tile([128, HChunk], f32, tag="zn")
                    zsrc = z_hb[h][:, ck * HChunk:(ck + 1) * HChunk]
                    nc.scalar.activation(
                        zn[:], zsrc, AF.Relu,
                        bias=b2[:, h:h + 1], scale=scal[:, 6:7])
                    nc.sync.dma_start(
                        y_d[b, h * 128:(h + 1) * 128, ck * HChunk:(ck + 1) * HChunk],
                        zn[:])

def _build_program():
    import concourse.bacc as bacc
    import concourse.tile as tile
    import concourse.mybir as mybir

    f32 = mybir.dt.float32
    bf16 = mybir.dt.bfloat16

    nc = bacc.Bacc("TRN2", target_bir_lowering=False, debug=False, num_devices=1)

    x_d = nc.dram_tensor("x", [CIN, B * SB_B], bf16, kind="ExternalInput").ap()
    w_d = nc.dram_tensor("w", [CIN, 2 * 9 * 128], bf16, kind="ExternalInput").ap()
    bq_d = nc.dram_tensor("bq", [128, 3], f32, kind="ExternalInput").ap()
    y_d = nc.dram_tensor("y", [B, COUT, HW], f32, kind="ExternalOutput").ap()

    with tile.TileContext(nc) as tc:
        _emit(nc, tc, x_d, w_d, bq_d, y_d)

    nc.compile()
    return nc


def _get_program():
    if "nc" not in _CACHE:
        _CACHE["nc"] = _build_program()
    return _CACHE["nc"]


def _binarize(t_np):
    """(t > t.mean()) as f32, matching the reference's jnp computation."""
    try:
        import jax.numpy as jnp
        tj = jnp.asarray(t_np)
        return np.asarray((tj > tj.mean()).astype(jnp.float32))
    except Exception:
        return (t_np > np.float32(t_np.astype(np.float64).mean())).astype(np.float32)


def kernel(x, weight, bias, train_mode=None):
    """Full-input entry point: shards over 8 NeuronCores, returns full output."""
    import time
    last_err = None
    for attempt in range(3):
        try:
            return _kernel_impl(x, weight, bias)
        except Exception as e:  # transient NRT/device errors: back off and retry
            last_err = e
            if attempt < 2:
                time.sleep(20.0 * (attempt + 1))
    raise last_err


def _kernel_impl(x, weight, bias):
    global LAST_RESULTS
    from concourse.bass_utils import run_bass_kernel_spmd

    x = np.asarray(x, dtype=np.float32)
    weight = np.asarray(weight, dtype=np.float32)
    bias = np.asarray(bias, dtype=np.float32)

    wq = _binarize(weight)                       # [256,128,3,3] {0,1}
    bq = _binarize(bias)                         # [256] {0,1}

    # weights -> lhsT layout [ci, (h,t,co_l)]
    wflat = np.ascontiguousarray(
        wq.reshape(2, 128, CIN, 9).transpose(2, 0, 3, 1).reshape(CIN, 2 * 9 * 128)
    ).astype(ml_dtypes.bfloat16)

    bq2 = np.zeros((128, 3), np.float32)
    bq2[:, 0] = bq[0:128]
    bq2[:, 1] = bq[128:256]
    bq2[:, 2] = HW * bq.sum()                    # C1 constant, replicated

    # x -> padded bf16 [b, ci, SB_B]
    xall = np.zeros((B_FULL, CIN, SB_B), dtype=ml_dtypes.bfloat16)
    xv = xall[:, :, :HP * HP].reshape(B_FULL, CIN, HP, HP)
    xv[:, :, 1:H + 1, 1:W + 1] = x.astype(ml_dtypes.bfloat16)

    in_maps = []
    for c in range(N_CORES):
        xc = np.ascontiguousarray(
            xall[c * B:(c + 1) * B].transpose(1, 0, 2).reshape(CIN, B * SB_B))
        in_maps.append({"x": xc, "w": wflat, "bq": bq2})

    nc = _get_program()
    kwargs = {}
    if TRACE:
        kwargs = dict(trace=True, tmpdir=TRACE_DIR)
    res = run_bass_kernel_spmd(nc, in_maps, core_ids=list(range(N_CORES)), **kwargs)
    LAST_RESULTS = res

    out = np.concatenate([res.results[c]["y"] for c in range(N_CORES)], axis=0)
    return out.reshape(B_FULL, COUT, H, W)

